# revision 20
# baseline (speedup 1.0000x reference)
"""DiT block kernel for Trainium2 (Bass/Tile), 8-core data-parallel.

Strategy:
  - Pure data parallelism: batch B=8, one batch element per NeuronCore, no
    collectives.
  - All transposes moved to the HOST: inputs arrive pre-transposed
    (xT [C,N] f32; qkv_wT [C,3C], proj_wT [C,C], fc1_wT [C,DFF],
    fc2_wT [DFF,C], ada_wT [C,6C] all BF16; channel-major bias mats
    [128, nch] f32); the device returns out^T [C,N] f32, which the host
    transposes back.
  - GEMM operands in BF16 (weights from host, activations y/qkT/z2/h_t/oT
    written bf16 on device); residual xres, LN statistics and modulation
    vectors stay f32.  PE rate is the same as f32r but DMA bytes halve and
    DVE gets 2x throughput on 16-bit.
  - adaLN GEMV done on PE with channel-major output accumulated directly in
    PSUM (ap=1 matmuls), silu(c) scattered channel-major once via DMA.
  - Causal attention computed as S^T = K^T-tiles @ Q^T with softmax row-sums
    obtained by augmenting V with a ones column; fully-masked tiles skipped,
    diagonal-straddling tiles masked post-exp.
"""

import sys

sys.path.insert(0, "/opt/trn_rl_repo")

import numpy as np

import concourse.bass as bass
import concourse.bacc as bacc
import concourse.mybir as mybir
from concourse import library_config
from concourse.tile import TileContext

F32 = mybir.dt.float32
F32R = mybir.dt.float32r
BF16 = mybir.dt.bfloat16
FP8 = mybir.dt.float8e4
DR = mybir.MatmulPerfMode.DoubleRow
AF = mybir.ActivationFunctionType
OP = mybir.AluOpType


def r(ap):
    return ap.bitcast(F32R)


def build_program(N=1024, C=1024, H=16, DFF=4096, head_group=8, dff_group=4,
                  n_cores=8, sim_safe=False, loop_iters=None, phase_cb=None,
                  use_fp8=True):
    MMD = FP8 if use_fp8 else BF16
    D = 64
    NT, CT, DT = N // 128, C // 128, DFF // 128
    NJ = N // 512
    HG = head_group
    NHG = H // HG
    G = dff_group
    NG = DT // G
    assert H % HG == 0 and DT % G == 0 and N % 512 == 0
    assert HG % 2 == 0 and D == 64

    nc = bacc.Bacc("TRN2", target_bir_lowering=False, debug=False,
                   num_devices=n_cores, num_swdge_queues=4)

    xT_d = nc.dram_tensor("xT", [C, N], F32, kind="ExternalInput")
    c_d = nc.dram_tensor("cvec", [C], F32, kind="ExternalInput")
    qkvwT_d = nc.dram_tensor("qkv_wT", [C, 3 * C], BF16, kind="ExternalInput")
    qkvb_d = nc.dram_tensor("qkv_b", [3 * C], F32, kind="ExternalInput")
    qkvbm_d = nc.dram_tensor("qkv_bm", [128, 3 * CT], F32, kind="ExternalInput")
    projwT_d = nc.dram_tensor("proj_wT", [C, C], MMD, kind="ExternalInput")
    projbm_d = nc.dram_tensor("proj_bm", [128, CT], F32, kind="ExternalInput")
    fc1wT_d = nc.dram_tensor("fc1_wT", [C, DFF], MMD, kind="ExternalInput")
    fc1bm_d = nc.dram_tensor("fc1_bm", [128, DT], F32, kind="ExternalInput")
    fc2wT_d = nc.dram_tensor("fc2_wT", [DFF, C], MMD, kind="ExternalInput")
    fc2bm_d = nc.dram_tensor("fc2_bm", [128, CT], F32, kind="ExternalInput")
    adawT_d = nc.dram_tensor("ada_wT", [C, 6 * C], BF16, kind="ExternalInput")
    adabm_d = nc.dram_tensor("ada_bm", [128, 6 * CT], F32, kind="ExternalInput")
    out_d = nc.dram_tensor("out", [C, N], F32, kind="ExternalOutput")

    qkvwT_r = qkvwT_d.ap().rearrange("(a p) j -> p a j", p=128)
    projwT_r = projwT_d.ap().rearrange("(a p) j -> p a j", p=128)
    fc1wT_r = fc1wT_d.ap().rearrange("(a p) j -> p a j", p=128)
    fc2wT_r = fc2wT_d.ap().rearrange("(a p) j -> p a j", p=128)
    adawT_r = adawT_d.ap().rearrange("(a p) j -> p a j", p=128)

    from contextlib import ExitStack
    with TileContext(nc) as tc, ExitStack() as ctx:
        consts = ctx.enter_context(tc.tile_pool(name="consts", bufs=1))
        sb = ctx.enter_context(tc.tile_pool(name="sb", bufs=1))
        ada_p = ctx.enter_context(tc.tile_pool(name="adap", bufs=2))
        wt_p = ctx.enter_context(tc.tile_pool(name="wt", bufs=3))
        pt_p = ctx.enter_context(tc.tile_pool(name="pt", bufs=3))
        rows_p = ctx.enter_context(tc.tile_pool(name="rows", bufs=3))
        bc_p = ctx.enter_context(tc.tile_pool(name="bc", bufs=2))
        sq_p = ctx.enter_context(tc.tile_pool(name="sqp", bufs=3))

        ps_mm = ctx.enter_context(
            tc.tile_pool(name="ps_mm", bufs=4, space="PSUM"))
        ps_row = ctx.enter_context(
            tc.tile_pool(name="ps_row", bufs=2, space="PSUM"))
        ps_o = ctx.enter_context(
            tc.tile_pool(name="ps_o", bufs=2, space="PSUM"))

        from contextlib import nullcontext
        loop_cm = tc.For_i(0, loop_iters, 1) if loop_iters else nullcontext()
        with loop_cm:
            _dmac = [0]

            def dma_rr(out, in_):
                i = _dmac[0]; _dmac[0] += 1
                eng = (nc.sync, nc.scalar, nc.gpsimd)[i % 3]
                eng.dma_start(out=out, in_=in_)

            # ================= standing tensors =================
            xres = sb.tile([128, CT, N], F32, tag="xres")
            for ci in range(CT):
                dma_rr(xres[:, ci, :], xT_d[ci * 128:(ci + 1) * 128, :])
            if phase_cb: phase_cb("x_in")

            # ================= constants =================
            ones65f = consts.tile([65, 128], F32, tag="ones65f")
            nc.vector.memset(ones65f, 1.0)
            ones65 = consts.tile([65, 128], F32R, tag="ones65")
            nc.vector.tensor_copy(ones65, ones65f)

            masks = consts.tile([128, 4, 512], BF16, tag="masks")
            nc.gpsimd.memset(masks, 1.0)
            for i in range(4):
                # keep where n >= m  <=>  s - r - delta >= 0 (delta = 128i)
                nc.gpsimd.affine_select(
                    out=masks[:, i, :], in_=masks[:, i, :], compare_op=OP.is_ge,
                    fill=0.0, base=-(128 * i), pattern=[[1, 512]],
                    channel_multiplier=-1)

            ones_invCf = consts.tile([128, 1], F32, tag="onescf")
            nc.gpsimd.memset(ones_invCf, 1.0 / C)
            ones_invC = consts.tile([128, 1], F32R, tag="onesc")
            nc.vector.tensor_copy(ones_invC, ones_invCf)
            eps_t = consts.tile([1, 1], F32, tag="eps")
            nc.vector.memset(eps_t, 1e-6)

            def bias_load(src_d, nch, tag):
                t = consts.tile([128, nch], F32, tag=tag)
                nc.sync.dma_start(out=t, in_=src_d[:, :])
                return t

            qkvb_t = bias_load(qkvbm_d, 3 * CT, "qkvbt")
            projb_t = bias_load(projbm_d, CT, "projbt")
            fc1b_t = bias_load(fc1bm_d, DT, "fc1bt")
            fc2b_t = bias_load(fc2bm_d, CT, "fc2bt")
            adab_t = bias_load(adabm_d, 6 * CT, "adabt")

            # ---- adaLN: silu(c), computed channel-major [128, CT] ----
            c_cm = consts.tile([128, CT], F32, tag="ccm")
            nc.sync.dma_start(out=c_cm,
                              in_=c_d.ap().rearrange("(b p) -> p b", p=128))
            silu_bT = consts.tile([128, CT], F32, tag="silubt")
            nc.scalar.activation(silu_bT, c_cm, AF.Sigmoid)
            nc.vector.tensor_mul(silu_bT, silu_bT, c_cm)
            silu_bTb = consts.tile([128, CT], BF16, tag="silubtb")
            nc.vector.tensor_copy(silu_bTb, silu_bT)

            modsb = consts.tile([128, 6 * CT], F32, tag="modsb")

            def ada_part(jts):
                # mod[p, jt] = sum_c ada_wT[c, jt*128+p] * silu(c), computed
                # channel-major directly in PSUM (ap=1 matmuls, bf16).
                lo, hi = min(jts), max(jts) + 1
                pmod = ps_row.tile([128, 512], F32, tag="row")
                for jb in range((hi - lo) // 4):      # chunks of 4 jt = 512 j
                    j0 = (lo + jb * 4) * 128
                    at = ada_p.tile([128, CT, 512], BF16, tag="ada")
                    dma_rr(at, adawT_r[:, :, j0:j0 + 512])
                    for k in range(4):
                        for ci in range(CT):
                            nc.tensor.matmul(
                                pmod[:, jb * 4 + k:jb * 4 + k + 1],
                                at[:, ci, k * 128:(k + 1) * 128],
                                silu_bTb[:, ci:ci + 1],
                                start=(ci == 0), stop=(ci == CT - 1))
                nc.vector.tensor_add(modsb[:, lo:hi], pmod[:, 0:hi - lo],
                                     adab_t[:, lo:hi])

            # LN1 statistics first: their PE/DVE work overlaps the ada1
            # weight streaming (engine queues execute in emission order).
            for nj in range(NJ):
                ln_stats(nj)
            # shift/scale_msa now (LN1 path); rest after attention is launched
            ada_part(range(0, 2 * CT))
            sp_msa = consts.tile([128, CT], F32, tag="spmsa")
            nc.vector.tensor_scalar(sp_msa, modsb[:, CT:2 * CT], 1.0, None, OP.add)
            if phase_cb: phase_cb("consts_ada1")

            # ================= helpers =================
            rbt = [consts.tile([128, 512], F32, tag="rb%d" % j)
                   for j in range(NJ)]
            mrbt = [consts.tile([128, 512], F32, tag="mrb%d" % j)
                    for j in range(NJ)]

            def ln_stats(nj):
                njs = slice(nj * 512, (nj + 1) * 512)
                xrb = sb.tile([128, CT, 512], F32R, tag="xrb")
                nc.scalar.activation(xrb, xres[:, :, njs], AF.Identity)
                s_mu = ps_row.tile([1, 512], F32, tag="row")
                for ci in range(CT):
                    nc.tensor.matmul(s_mu, ones_invC, xrb[:, ci, :],
                                     start=(ci == 0), stop=(ci == CT - 1))
                sqb = sb.tile([128, CT, 512], F32R, tag="sqb")
                nc.vector.tensor_mul(sqb, xrb, xrb)
                s_sq = ps_row.tile([1, 512], F32, tag="row")
                for ci in range(CT):
                    nc.tensor.matmul(s_sq, ones_invC, sqb[:, ci, :],
                                     start=(ci == 0), stop=(ci == CT - 1))
                t_mu = rows_p.tile([1, 512], F32, tag="rows")
                nc.vector.tensor_copy(t_mu, s_mu)
                t_var = rows_p.tile([1, 512], F32R, tag="rows")
                nc.vector.tensor_mul(t_var, t_mu, t_mu)
                nc.vector.tensor_sub(t_var, s_sq, t_var)
                t_rstd = rows_p.tile([1, 512], F32R, tag="rows")
                nc.scalar.activation(t_rstd, t_var, AF.Sqrt, bias=eps_t)
                with nc.allow_low_precision(reason="f32r rstd"):
                    nc.vector.reciprocal(t_var, t_rstd)      # t_var = rstd
                nc.vector.tensor_mul(t_rstd, t_mu, t_var)    # mu * rstd
                bc_r = ps_row.tile([128, 512], F32, tag="row")
                nc.tensor.matmul(bc_r, ones65[0:1, :], t_var[0:1, :],
                                 start=True, stop=True)
                bc_mr = ps_row.tile([128, 512], F32, tag="row")
                nc.tensor.matmul(bc_mr, ones65[0:1, :], t_rstd[0:1, :],
                                 start=True, stop=True)
                nc.vector.tensor_copy(rbt[nj], bc_r)
                nc.vector.tensor_copy(mrbt[nj], bc_mr)

            def ln_mod(dst, nj, sh_off, sp_tile):
                njs = slice(nj * 512, (nj + 1) * 512)
                for ci in range(CT):
                    t = dst[:, ci, njs]
                    nc.vector.tensor_mul(t, xres[:, ci, njs], rbt[nj])
                    nc.vector.tensor_sub(t, t, mrbt[nj])
                    nc.scalar.activation(
                        t, t, AF.Identity,
                        scale=sp_tile[:, ci:ci + 1],
                        bias=modsb[:, sh_off + ci:sh_off + ci + 1])

            # ================= attention =================
            y = sb.tile([128, CT, N], BF16, tag="lnout")
            for nj in range(NJ):
                ln_mod(y, nj, 0, sp_msa)
            if phase_cb: phase_cb("ln1")

            oT = sb.tile([128, CT, N], MMD, tag="oT")

            for g in range(NHG):
                # ---- q,k (channel-major) ----
                qkT = sb.tile([128, HG, N], BF16, tag="big")
                for half in range(2):           # 0 = q, 1 = k
                    base_col = half * C + g * (HG * D)
                    for chunk in range(2):      # 256 cols each
                        wqk = wt_p.tile([128, CT, 256], BF16, tag="wt")
                        col0 = base_col + chunk * 256
                        dma_rr(wqk, qkvwT_r[:, :, col0:col0 + 256])
                        for s2 in range(2):
                            sl = half * (HG // 2) + chunk * 2 + s2
                            fi = (half * CT + g * (HG // 2)
                                  + chunk * 2 + s2)
                            for nj in range(NJ):
                                njs = slice(nj * 512, (nj + 1) * 512)
                                pmm = ps_mm.tile([128, 512], F32, tag="mm")
                                for ci in range(CT):
                                    nc.tensor.matmul(
                                        pmm,
                                        wqk[:, ci, s2 * 128:(s2 + 1) * 128],
                                        y[:, ci, njs],
                                        start=(ci == 0), stop=(ci == CT - 1))
                                nc.scalar.activation(
                                    qkT[:, sl, njs], pmm, AF.Identity,
                                    bias=qkvb_t[:, fi:fi + 1])
                if phase_cb: phase_cb("qk_gemm")

                if g == 0:
                    # remaining modulation vectors; overlaps attention compute
                    ada_part(range(2 * CT, 6 * CT))
                    sp_mlp = consts.tile([128, CT], F32, tag="spmlp")
                    nc.vector.tensor_scalar(sp_mlp, modsb[:, 4 * CT:5 * CT],
                                            1.0, None, OP.add)
                    bg1 = consts.tile([128, CT], F32, tag="bg1")
                    nc.vector.tensor_mul(bg1, modsb[:, 2 * CT:3 * CT], projb_t)
                    bg2 = consts.tile([128, CT], F32, tag="bg2")
                    nc.vector.tensor_mul(bg2, modsb[:, 5 * CT:6 * CT], fc2b_t)
                    if phase_cb: phase_cb("ada2")

                # ---- v (token-major, ones-augmented) ----
                vaug = sb.tile([128, NT, HG, 65], BF16, tag="vaug")
                nc.gpsimd.memset(vaug[:, :, :, 64:65], 1.0)
                for vg in range(2):
                    wv = wt_p.tile([128, CT, 256], BF16, tag="wt")
                    col0 = 2 * C + g * (HG * D) + vg * 256
                    dma_rr(wv, qkvwT_r[:, :, col0:col0 + 256])
                    vbrow = rows_p.tile([1, 512], F32R, tag="rows")
                    off = 2 * C + (g * HG + vg * 4) * 64
                    nc.sync.dma_start(
                        out=vbrow[0:1, 0:256],
                        in_=r(qkvb_d[off:off + 256]).rearrange("(a c) -> a c",
                                                               a=1))
                    vbp = ps_row.tile([128, 512], F32, tag="row")
                    nc.tensor.matmul(vbp[:, 0:256], ones65[0:1, :],
                                     vbrow[0:1, 0:256], start=True, stop=True)
                    vb = bc_p.tile([128, 512], F32, tag="bc")
                    nc.vector.tensor_copy(vb[:, 0:256], vbp[:, 0:256])
                    for ni in range(NT):
                        pv = ps_mm.tile([128, 512], F32, tag="mm")
                        for ci in range(CT):
                            nc.tensor.matmul(
                                pv[:, 0:256],
                                y[:, ci, ni * 128:(ni + 1) * 128],
                                wv[:, ci, :],
                                start=(ci == 0), stop=(ci == CT - 1))
                        nc.vector.tensor_add(
                            vaug[:, ni, vg * 4:vg * 4 + 4, 0:64],
                            pv[:, 0:256].rearrange("p (a b) -> p a b", a=4),
                            vb[:, 0:256].rearrange("p (a b) -> p a b", a=4))
                if phase_cb: phase_cb("v_gemm")

                # ---- attention proper ----
                for nj in range(NJ):
                    njs = slice(nj * 512, (nj + 1) * 512)
                    mi_hi = min(NT, 4 * (nj + 1))
                    for hp in range(HG // 2):
                        qsl, ksl = hp, HG // 2 + hp
                        po0 = ps_o.tile([65, 512], F32, tag="po")
                        po1 = ps_o.tile([65, 512], F32, tag="po")
                        pos = [po0, po1]
                        for mi in range(mi_hi):
                            delta = 128 * mi - 512 * nj
                            lo = max(delta, 0)
                            los = slice(lo, 512)
                            qlos = slice(nj * 512 + lo, (nj + 1) * 512)
                            pts = []
                            sps = []
                            for sub in range(2):
                                base = sub * 64
                                ps_s = ps_mm.tile([128, 512], F32,
                                                  tag="mm", name="s%d" % sub)
                                # pair shares PE via disjoint row groups
                                nc.tensor.matmul(
                                    ps_s[:, los],
                                    qkT[base:base + 64, ksl,
                                        mi * 128:(mi + 1) * 128],
                                    qkT[base:base + 64, qsl, qlos],
                                    start=True, stop=True)
                                sps.append(ps_s)
                            for sub in range(2):
                                pt = pt_p.tile([128, 512], BF16,
                                               tag="pt", name="pt%d" % sub)
                                nc.scalar.activation(pt[:, los],
                                                     sps[sub][:, los],
                                                     AF.Exp, scale=D ** -0.5)
                                if delta >= 0:
                                    band = min(128, 512 - delta)
                                    nc.vector.tensor_mul(
                                        pt[:, delta:delta + band],
                                        pt[:, delta:delta + band],
                                        masks[:, delta // 128,
                                              delta:delta + band])
                                pts.append(pt)
                            for sub in range(2):
                                hl = 2 * hp + sub
                                nc.tensor.matmul(pos[sub][:, los],
                                                 vaug[:, mi, hl, :],
                                                 pts[sub][:, los],
                                                 start=(mi == 0),
                                                 stop=(mi == mi_hi - 1))
                        for sub in range(2):
                            hl = 2 * hp + sub
                            h_glob = g * HG + hl
                            po = pos[sub]
                            srow = rows_p.tile([65, 512], F32R, tag="rows")
                            with nc.allow_low_precision(reason="f32r recip"):
                                nc.vector.reciprocal(srow[64:65, :],
                                                     po[64:65, :])
                            rbp = ps_row.tile([128, 512], F32, tag="row")
                            nc.tensor.matmul(rbp[:, :], ones65[64:65, :],
                                             srow[64:65, :],
                                             start=True, stop=True)
                            rb = bc_p.tile([128, 512], F32, tag="bc")
                            nc.vector.tensor_copy(rb[0:64, :], rbp[0:64, :])
                            if sub == 0:
                                nc.vector.tensor_mul(
                                    oT[0:64, h_glob // 2, njs],
                                    po[0:64, :], rb[0:64, :])
                            else:
                                tsh = pt_p.tile([128, 512], MMD,
                                               name="tshf", tag="pt")
                                nc.vector.tensor_mul(tsh[0:64, :],
                                                     po[0:64, :],
                                                     rb[0:64, :])
                                nc.sync.dma_start(
                                    out=oT[64:128, h_glob // 2, njs],
                                    in_=tsh[0:64, :])
            if phase_cb: phase_cb("attn_sm")

            # ================= proj + gated residual =================
            wp_all = sb.tile([128, CT, C], BF16, tag="sqb")
            dma_rr(wp_all, projwT_r[:, :, :])
            for nj in range(NJ):
                njs = slice(nj * 512, (nj + 1) * 512)
                for jc in range(CT):
                    pmm = ps_mm.tile([128, 512], F32, tag="mm")
                    if use_fp8:
                        for c2 in range(CT // 2):
                            nc.tensor.matmul(
                                pmm,
                                wp_all[:, 2 * c2:2 * c2 + 2,
                                       jc * 128:(jc + 1) * 128],
                                oT[:, 2 * c2:2 * c2 + 2, njs],
                                start=(c2 == 0),
                                stop=(c2 == CT // 2 - 1), perf_mode=DR)
                    else:
                        for ci in range(CT):
                            nc.tensor.matmul(
                                pmm, wp_all[:, ci, jc * 128:(jc + 1) * 128],
                                oT[:, ci, njs],
                                start=(ci == 0), stop=(ci == CT - 1))
                    t = sq_p.tile([128, 512], F32, tag="sq")
                    nc.vector.tensor_scalar(
                        t, pmm, modsb[:, 2 * CT + jc:2 * CT + jc + 1],
                        bg1[:, jc:jc + 1], OP.mult, OP.add)
                    nc.vector.tensor_add(xres[:, jc, njs],
                                         xres[:, jc, njs], t)
                ln_stats(nj)
            if phase_cb: phase_cb("proj")

            # ================= MLP =================
            z2 = sb.tile([128, CT, N], MMD, tag="lnout")
            for nj in range(NJ):
                ln_mod(z2, nj, 3 * CT, sp_mlp)
            if phase_cb: phase_cb("ln2")

            for nj in range(NJ):
                njs = slice(nj * 512, (nj + 1) * 512)
                h_nj = sb.tile([128, DT, 512], MMD, tag="big",
                               name="hnj%d" % nj)
                for wc in range(DT // 2):     # 16 chunks of 256 fc1 cols
                    w1 = wt_p.tile([128, CT, 256], MMD, tag="wt")
                    dma_rr(w1, fc1wT_r[:, :, wc * 256:(wc + 1) * 256])
                    for s2 in range(2):
                        dd = wc * 2 + s2
                        ph = ps_mm.tile([128, 512], F32, tag="mm")
                        if use_fp8:
                            for c2 in range(CT // 2):
                                nc.tensor.matmul(
                                    ph,
                                    w1[:, 2 * c2:2 * c2 + 2,
                                       s2 * 128:(s2 + 1) * 128],
                                    z2[:, 2 * c2:2 * c2 + 2, njs],
                                    start=(c2 == 0),
                                    stop=(c2 == CT // 2 - 1), perf_mode=DR)
                        else:
                            for ci in range(CT):
                                nc.tensor.matmul(
                                    ph, w1[:, ci, s2 * 128:(s2 + 1) * 128],
                                    z2[:, ci, njs],
                                    start=(ci == 0), stop=(ci == CT - 1))
                        if not sim_safe:
                            nc.scalar.activation(h_nj[:, dd, :], ph,
                                                 AF.Gelu_apprx_tanh,
                                                 bias=fc1b_t[:, dd:dd + 1])
                        else:
                            # gelu_tanh decomposed for CoreSim
                            s2pi = float(np.sqrt(2.0 / np.pi))
                            hs = sq_p.tile([128, 512], F32, tag="sq")
                            nc.scalar.activation(hs, ph, AF.Identity,
                                                 bias=fc1b_t[:, dd:dd + 1])
                            hq = sq_p.tile([128, 512], F32, tag="sq")
                            nc.scalar.activation(hq, ph, AF.Square,
                                                 bias=fc1b_t[:, dd:dd + 1])
                            nc.vector.tensor_scalar(hq, hq,
                                                    s2pi * 0.044715,
                                                    s2pi, OP.mult, OP.add)
                            nc.vector.tensor_mul(hq, hq, hs)
                            nc.scalar.activation(hq, hq, AF.Tanh)
                            nc.vector.tensor_scalar(hq, hq, 0.5, 0.5,
                                                    OP.mult, OP.add)
                            nc.vector.tensor_mul(h_nj[:, dd, :], hq, hs)
                # fc2: full-depth PSUM accumulation, w2 slices double-buffered
                for jc in range(CT):
                    w2j = ada_p.tile([128, DT, 128], MMD, tag="ada")
                    dma_rr(w2j, fc2wT_r[:, :, jc * 128:(jc + 1) * 128])
                    pm = ps_mm.tile([128, 512], F32, tag="mm")
                    if use_fp8:
                        for t2 in range(DT // 2):
                            nc.tensor.matmul(
                                pm, w2j[:, 2 * t2:2 * t2 + 2, :],
                                h_nj[:, 2 * t2:2 * t2 + 2, :],
                                start=(t2 == 0), stop=(t2 == DT // 2 - 1),
                                perf_mode=DR)
                    else:
                        for dl in range(DT):
                            nc.tensor.matmul(
                                pm, w2j[:, dl, :], h_nj[:, dl, :],
                                start=(dl == 0), stop=(dl == DT - 1))
                    t = sq_p.tile([128, 512], F32, tag="sq")
                    nc.vector.tensor_scalar(
                        t, pm, modsb[:, 5 * CT + jc:5 * CT + jc + 1],
                        bg2[:, jc:jc + 1], OP.mult, OP.add)
                    nc.vector.tensor_add(xres[:, jc, njs],
                                         xres[:, jc, njs], t)
                    dma_rr(out_d[jc * 128:(jc + 1) * 128, njs],
                           xres[:, jc, njs])
            if phase_cb: phase_cb("fc2")

    if not nc.is_finalized():
        nc.finalize()
    return nc


_CACHE = {}


def get_program(key="full", **kw):
    if key not in _CACHE:
        _CACHE[key] = build_program(**kw)
    return _CACHE[key]


def _bias_mat(b, nch):
    return np.ascontiguousarray(
        np.asarray(b, np.float32).reshape(nch, 128).T)


USE_FP8 = False


def make_in_maps(inputs, use_fp8=None):
    import ml_dtypes
    BF = ml_dtypes.bfloat16
    F8 = ml_dtypes.float8_e4m3 if (USE_FP8 if use_fp8 is None else use_fp8) \
        else ml_dtypes.bfloat16
    x = np.ascontiguousarray(np.asarray(inputs["x"], np.float32))
    c = np.ascontiguousarray(np.asarray(inputs["c"], np.float32))
    B, N, C = x.shape
    CT = C // 128
    DFF = 4 * C
    DT = DFF // 128
    f32 = np.float32
    qkv_w = np.asarray(inputs["qkv_w"], f32)
    proj_w = np.asarray(inputs["proj_w"], f32)
    fc1_w = np.asarray(inputs["fc1_w"], f32)
    fc2_w = np.asarray(inputs["fc2_w"], f32)
    ada_w = np.asarray(inputs["ada_w"], f32)
    shared = {
        "qkv_wT": np.ascontiguousarray(qkv_w.T.astype(BF)),
        "qkv_b": np.ascontiguousarray(np.asarray(inputs["qkv_b"], f32)),
        "qkv_bm": _bias_mat(inputs["qkv_b"], 3 * CT),
        "proj_wT": np.ascontiguousarray(proj_w.T.astype(F8)),
        "proj_bm": _bias_mat(inputs["proj_b"], CT),
        "fc1_wT": np.ascontiguousarray(fc1_w.T.astype(F8)),
        "fc1_bm": _bias_mat(inputs["fc1_b"], DT),
        "fc2_wT": np.ascontiguousarray(fc2_w.T.astype(F8)),
        "fc2_bm": _bias_mat(inputs["fc2_b"], CT),
        "ada_wT": np.ascontiguousarray(ada_w.T.astype(BF)),
        "ada_bm": _bias_mat(inputs["ada_b"], 6 * CT),
    }
    return [dict(shared, xT=np.ascontiguousarray(x[b].T), cvec=c[b, 0])
            for b in range(B)]


def kernel(**inputs):
    from concourse.bass_utils import run_bass_kernel_spmd

    x = np.asarray(inputs["x"])
    B, N, C = x.shape
    nc = get_program("full", N=N, C=C, H=16, DFF=4 * C, head_group=8,
                     dff_group=4, n_cores=B, use_fp8=USE_FP8)
    in_maps = make_in_maps(inputs)
    res = run_bass_kernel_spmd(nc, in_maps, core_ids=list(range(B)))
    out = np.stack([res.results[b]["out"].T for b in range(B)], axis=0)
    return np.ascontiguousarray(out).astype(np.float32)
